# revision 8
# baseline (speedup 1.0000x reference)
"""GaussianImage (Cholesky) renderer on 8 trn2 NeuronCores.

Strategy: tile-parallel over the pixel grid with slot packing.  The
256x256 image is cut into 16x16-pixel tiles.  The host does all the
per-gaussian math (tanh / sigmoid / conic / quadratic-form coefficients,
in float64) and bins gaussians to the tiles they can reach (conservative
support radius; beyond it exp(-sigma) < exp(-CUT) is dropped).  Each
(gaussian, tile) pair is one SLOT; slots pack 128-per-bin across up to
GMAX tiles.  Key fact: every tile uses the SAME local pixel window, so
sigma for a whole bin is ONE K=9 matmul against a shared 9x256 basis:

  sigma[s,p] = 0.25*gx2[p] - Ar[s]*gx2[p] + B[s]*gxy[p] + 0.25*gy2[p]
             - Cr[s]*gy2[p] + Di[s]*gx[p] + Df[s]*gx[p] + Ei[s]*gy[p]
             + Ef[s]*gy[p]

with gx,gy integer-centered (exact in bf16), quadratic coefs split as
0.25 - residual and linear coefs split int+frac, so the bf16 matmul
carries < 0.02 absolute error in sigma.  The constant term F of each
quadratic form is folded into the opacity host-side
(w' = color * opac * exp(-F)); |F| <= ~82 keeps exponents in range.

  alpha = Exp(-sigma)            [ScalarE, -> bf16]
  img   = W'^T @ alpha           [TensorE bf16, W' slot-block columns]
  out   = clamp(img, 0, 1)       [VectorE, also the PSUM->SBUF move]

All matmuls are bf16.  All device inputs ride ONE packed DMA; all bin
outputs leave in ONE DMA (instruction/semaphore count dominates the
runtime at this size, not FLOPs or bytes).  Each pixel is owned by
exactly one tile -> no cross-core reduction.  Per core: NG(=3) bins.
"""

import os
import numpy as np
import ml_dtypes

T, N, H, W = 2, 512, 256, 256
TILE = 16
NT = H // TILE            # 16 tiles per axis
PIX = TILE * TILE         # 256
N_CORES = 8
SLOTS = 128               # gaussian-slot capacity of one bin (partition dim)
GMAX = 25                 # max tiles per bin (3 color rows each -> WD = 75)
KB = 9                    # basis rows
WD = 3 * GMAX             # 75 output color rows per bin
CUT = 4.0                 # sigma cutoff for binning

_CACHE = {}


def _build_nc(NG, WDs):
    import concourse.bass as bass
    import concourse.mybir as mybir
    from concourse.tile import TileContext
    import bass_rust

    f32 = mybir.dt.float32
    bf16 = mybir.dt.bfloat16
    Alu = mybir.AluOpType
    Act = mybir.ActivationFunctionType

    CC = PIX + NG * SLOTS             # coef-tensor columns (basis + per-bin lhsT)
    nc = bass.Bass("TRN2")
    coef_d = nc.dram_tensor("coef", [KB, CC], bf16, kind="ExternalInput")
    w_d = nc.dram_tensor("wmat", [SLOTS, NG * WD], bf16, kind="ExternalInput")
    out_d = nc.dram_tensor("out", [WD, NG * PIX], bf16, kind="ExternalOutput")

    with TileContext(nc) as tc:
        with tc.tile_pool(name="const", bufs=1) as cpool, \
             tc.tile_pool(name="alpha", bufs=3) as apool, \
             tc.tile_pool(name="stg", bufs=3) as spool, \
             tc.tile_pool(name="ps_sig", bufs=3, space="PSUM") as psig, \
             tc.tile_pool(name="ps_img", bufs=2, space="PSUM") as pimg:

            # trigger the exp ACT-table load (~2.7us) immediately so it
            # overlaps the input DMA instead of serializing after it
            warm = cpool.tile([SLOTS, 1], f32, tag="warm")
            nc.gpsimd.memset(warm, 0.0)
            nc.scalar.activation(warm, warm, Act.Exp)

            cf = cpool.tile([KB, CC], bf16, tag="coef")
            wt = cpool.tile([SLOTS, NG, WD], bf16, tag="wmat")
            nc.sync.dma_start(out=cf, in_=coef_d[:])
            nc.sync.dma_start(out=wt, in_=w_d[:].rearrange("p (g w) -> p g w", w=WD))
            bt = cf[:, 0:PIX]

            sigs = []
            for g in range(NG):
                lh = cf[:, PIX + g * SLOTS:PIX + (g + 1) * SLOTS]
                sig = psig.tile([SLOTS, PIX], f32, tag="sig", name=f"sig{g}")
                nc.tensor.matmul(sig, lh, bt, start=True, stop=True)
                sigs.append(sig)
            for g in range(NG):
                alpha = apool.tile([SLOTS, PIX], bf16, tag="alpha")
                nc.scalar.activation(alpha, sigs[g], Act.Exp, scale=-1.0)
                wd = WDs[g]
                img = pimg.tile([wd, PIX], f32, tag="img", name=f"img{g}")
                nc.tensor.matmul(img, wt[:, g, 0:wd], alpha, start=True, stop=True)
                st = spool.tile([wd, PIX], bf16, tag="st", name=f"st{g}")
                nc.vector.tensor_scalar(out=st, in0=img, scalar1=0.0,
                                        scalar2=1.0, op0=Alu.max, op1=Alu.min)
                eng = nc.scalar if g == 1 else nc.sync
                eng.dma_start(out=out_d[0:wd, g * PIX:(g + 1) * PIX], in_=st)

    bass_rust.generate_event_semaphores(nc)
    return nc


def _plan(cx, cy, lam):
    """Tile binning + first-fit-decreasing slot packing (host side)."""
    r = np.sqrt(2.0 * CUT * np.maximum(lam, 0.0)) + 1.0
    tiles = []  # (t, ty, tx, index-array)
    for t in range(T):
        x0 = np.clip(((cx[t] - r[t]) // TILE).astype(int), 0, NT - 1)
        x1 = np.clip(((cx[t] + r[t]) // TILE).astype(int), 0, NT - 1)
        y0 = np.clip(((cy[t] - r[t]) // TILE).astype(int), 0, NT - 1)
        y1 = np.clip(((cy[t] + r[t]) // TILE).astype(int), 0, NT - 1)
        buckets = [[[] for _ in range(NT)] for _ in range(NT)]
        for n in range(N):
            for ty in range(y0[n], y1[n] + 1):
                for tx in range(x0[n], x1[n] + 1):
                    buckets[ty][tx].append(n)
        for ty in range(NT):
            for tx in range(NT):
                if buckets[ty][tx]:
                    assert len(buckets[ty][tx]) <= SLOTS
                    tiles.append((t, ty, tx, np.asarray(buckets[ty][tx])))

    tiles.sort(key=lambda e: -len(e[3]))
    bins = []  # [slots_used, [(t,ty,tx,idxs,slot_off), ...]]
    for t, ty, tx, idxs in tiles:
        n = len(idxs)
        for b in bins:
            if b[0] + n <= SLOTS and len(b[1]) < GMAX:
                b[1].append((t, ty, tx, idxs, b[0]))
                b[0] += n
                break
        else:
            bins.append([n, [(t, ty, tx, idxs, 0)]])
    return bins


def _ensure_ntff_hook():
    """Provide antenv.axon_hooks (missing in this image) so trace=True works."""
    import sys, types, ctypes, contextlib
    if "antenv.axon_hooks" in sys.modules:
        return
    so_path = "/opt/axon/libaxon_pjrt.so"
    if not os.path.exists(so_path):
        return
    lib = ctypes.CDLL(so_path)
    if not hasattr(lib, "axon_start_nrt_profile"):
        return
    lib.axon_start_nrt_profile.argtypes = [ctypes.POINTER(ctypes.c_int64), ctypes.c_size_t]
    lib.axon_start_nrt_profile.restype = ctypes.c_int64
    lib.axon_stop_nrt_profile.argtypes = [ctypes.c_char_p]
    lib.axon_stop_nrt_profile.restype = ctypes.c_int64

    import contextlib

    @contextlib.contextmanager
    def _hook(output_dir, device_ids):
        import jax
        jax.devices()
        if device_ids:
            ids = (ctypes.c_int64 * len(device_ids))(*device_ids)
            rc = lib.axon_start_nrt_profile(ids, len(device_ids))
        else:
            rc = lib.axon_start_nrt_profile(None, 0)
        if rc != 0:
            raise RuntimeError(f"axon_start_nrt_profile rc={rc}")
        try:
            yield
        finally:
            n = lib.axon_stop_nrt_profile(str(output_dir).encode())
            print(f"profile: {n} file(s) written to {output_dir}")

    mod = types.ModuleType("antenv.axon_hooks")
    mod.get_axon_ntff_profile_hook = lambda: _hook
    mod.set_axon_ntff_profile_hook = lambda h: None
    sys.modules["antenv.axon_hooks"] = mod


def kernel(xyz, cholesky, opacity, features_dc):
    from concourse import bass_utils

    xyz = np.asarray(xyz, np.float32)
    cholesky = np.asarray(cholesky, np.float32)
    opacity = np.asarray(opacity, np.float32)
    features_dc = np.asarray(features_dc, np.float32)

    # host-side gaussian math, float64
    means = np.tanh(xyz.astype(np.float64))
    cx = 0.5 * W * (means[..., 0] + 1.0)
    cy = 0.5 * H * (means[..., 1] + 1.0)
    chol = cholesky.astype(np.float64) + np.array([0.5, 0.0, 0.5])
    l0, l1, l2 = chol[..., 0], chol[..., 1], chol[..., 2]
    sxx, sxy, syy = l0 * l0, l0 * l1, l1 * l1 + l2 * l2
    det = sxx * syy - sxy * sxy
    ca, cb, cc = syy / det, -sxy / det, sxx / det
    tr = sxx + syy
    lam = tr / 2 + np.sqrt(np.maximum(tr * tr / 4 - det, 0.0))
    colors = 1.0 / (1.0 + np.exp(-features_dc.astype(np.float64)))   # (N,3)
    opac = (1.0 / (1.0 + np.exp(-opacity.astype(np.float64))))[:, 0]  # (N,)

    bins = _plan(cx, cy, lam)
    bins.sort(key=lambda b: -len(b[1]))   # largest tile-count first
    B = len(bins)
    NG = (B + N_CORES - 1) // N_CORES
    CC = PIX + NG * SLOTS
    # per-slot output widths: slot g serves bins[g*8 .. g*8+7] (dealt below)
    WDs = tuple(3 * max((len(bins[g * N_CORES + c][1])
                         for c in range(N_CORES) if g * N_CORES + c < B), default=1)
                for g in range(NG))

    # shared basis: tile-local integer-centered pixel coords (exact in bf16)
    c0 = float(TILE // 2)                                   # 8.0
    gxl = ((np.arange(PIX) % TILE) - c0).astype(np.float64)
    gyl = ((np.arange(PIX) // TILE) - c0).astype(np.float64)
    basis = np.stack([gxl * gxl, gxl * gxl, gxl * gyl, gyl * gyl, gyl * gyl,
                      gxl, gxl, gyl, gyl])                  # (9,PIX)

    in_maps = []
    meta = []  # per core: per bin: list of (t,ty,tx,e)
    for c in range(N_CORES):
        coef = np.zeros((KB, CC), np.float64)
        coef[:, 0:PIX] = basis
        wmat = np.zeros((SLOTS, NG * WD), np.float64)
        core_meta = []
        for g in range(NG):
            k = g * N_CORES + c
            ents = []
            if k < B:
                for e, (t, ty, tx, idxs, off) in enumerate(bins[k][1]):
                    n = len(idxs)
                    ex = cx[t, idxs] - (tx * TILE + c0)
                    ey = cy[t, idxs] - (ty * TILE + c0)
                    A_ = 0.5 * ca[t, idxs]
                    B_ = cb[t, idxs]
                    C_ = 0.5 * cc[t, idxs]
                    D_ = -(ca[t, idxs] * ex + cb[t, idxs] * ey)
                    E_ = -(cb[t, idxs] * ex + cc[t, idxs] * ey)
                    F_ = (0.5 * ca[t, idxs] * ex * ex + cb[t, idxs] * ex * ey
                          + 0.5 * cc[t, idxs] * ey * ey)
                    Di, Ei = np.round(D_), np.round(E_)
                    sl = slice(PIX + g * SLOTS + off, PIX + g * SLOTS + off + n)
                    coef[:, sl] = np.stack([
                        np.full(n, 0.25), -(0.25 - A_), B_,
                        np.full(n, 0.25), -(0.25 - C_),
                        Di, D_ - Di, Ei, E_ - Ei])
                    wmat[off:off + n, g * WD + 3 * e:g * WD + 3 * e + 3] = \
                        colors[idxs] * (opac[idxs] * np.exp(-F_))[:, None]
                    ents.append((t, ty, tx, e))
            core_meta.append(ents)
        in_maps.append({"coef": coef.astype(ml_dtypes.bfloat16),
                        "wmat": wmat.astype(ml_dtypes.bfloat16)})
        meta.append(core_meta)

    key = (NG, WDs)
    if key not in _CACHE:
        _CACHE[key] = _build_nc(NG, WDs)
    nc = _CACHE[key]

    trace = bool(int(os.environ.get("GS_TRACE", "0")))
    if trace:
        _ensure_ntff_hook()
    res = bass_utils.run_bass_kernel_spmd(
        nc, in_maps, core_ids=list(range(N_CORES)), trace=trace)
    kernel.last_result = res

    img = np.zeros((T, 3, H, W), np.float32)
    for c in range(N_CORES):
        o = np.asarray(res.results[c]["out"]).astype(np.float32).reshape(WD, NG, PIX)
        for g, ents in enumerate(meta[c]):
            for (t, ty, tx, e) in ents:
                img[t, :, ty * TILE:(ty + 1) * TILE, tx * TILE:(tx + 1) * TILE] = \
                    o[3 * e:3 * e + 3, g].reshape(3, TILE, TILE)
    return img


# revision 9
# speedup vs baseline: 1.0178x; 1.0178x over previous
"""GaussianImage (Cholesky) renderer on 8 trn2 NeuronCores.

Strategy: tile-parallel over the pixel grid with slot packing.  The
256x256 image is cut into 16x16-pixel tiles.  The host does all the
per-gaussian math (tanh / sigmoid / conic / quadratic-form coefficients,
in float64) and bins gaussians to the tiles they can reach (conservative
support radius; beyond it exp(-sigma) < exp(-CUT) is dropped).  Each
(gaussian, tile) pair is one SLOT; slots pack 128-per-bin across up to
GMAX tiles.  Key fact: every tile uses the SAME local pixel window, so
sigma for a whole bin is ONE K=9 matmul against a shared 9x256 basis:

  sigma[s,p] = 0.25*gx2[p] - Ar[s]*gx2[p] + B[s]*gxy[p] + 0.25*gy2[p]
             - Cr[s]*gy2[p] + Di[s]*gx[p] + Df[s]*gx[p] + Ei[s]*gy[p]
             + Ef[s]*gy[p]

with gx,gy integer-centered (exact in bf16), quadratic coefs split as
0.25 - residual and linear coefs split int+frac, so the bf16 matmul
carries < 0.02 absolute error in sigma.  The constant term F of each
quadratic form is folded into the opacity host-side
(w' = color * opac * exp(-F)); |F| <= ~82 keeps exponents in range.

  alpha = Exp(-sigma)            [ScalarE, -> bf16]
  img   = W'^T @ alpha           [TensorE bf16, W' slot-block columns]
  out   = clamp(img, 0, 1)       [VectorE, also the PSUM->SBUF move]

All matmuls are bf16.  All device inputs ride ONE packed DMA; all bin
outputs leave in ONE DMA (instruction/semaphore count dominates the
runtime at this size, not FLOPs or bytes).  Each pixel is owned by
exactly one tile -> no cross-core reduction.  Per core: NG(=3) bins.
"""

import os
import numpy as np
import ml_dtypes

T, N, H, W = 2, 512, 256, 256
TILE = 16
NT = H // TILE            # 16 tiles per axis
PIX = TILE * TILE         # 256
N_CORES = 8
SLOTS = 128               # gaussian-slot capacity of one bin (partition dim)
GMAX = 25                 # max tiles per bin (3 color rows each -> WD = 75)
KB = 9                    # basis rows
WD = 3 * GMAX             # 75 output color rows per bin
CUT = 4.0                 # sigma cutoff for binning

_CACHE = {}


def _build_nc(NG, WDs):
    import concourse.bass as bass
    import concourse.mybir as mybir
    from concourse.tile import TileContext
    import bass_rust

    f32 = mybir.dt.float32
    bf16 = mybir.dt.bfloat16
    Alu = mybir.AluOpType
    Act = mybir.ActivationFunctionType

    CC = PIX + NG * SLOTS             # coef-tensor columns (basis + per-bin lhsT)
    nc = bass.Bass("TRN2")
    coef_d = nc.dram_tensor("coef", [KB, CC], bf16, kind="ExternalInput")
    w_d = nc.dram_tensor("wmat", [SLOTS, NG * WD], bf16, kind="ExternalInput")
    out_d = nc.dram_tensor("out", [WD, NG * PIX], bf16, kind="ExternalOutput")

    with TileContext(nc) as tc:
        with tc.tile_pool(name="const", bufs=1) as cpool, \
             tc.tile_pool(name="alpha", bufs=3) as apool, \
             tc.tile_pool(name="stg", bufs=3) as spool, \
             tc.tile_pool(name="ps_sig", bufs=3, space="PSUM") as psig, \
             tc.tile_pool(name="ps_img", bufs=2, space="PSUM") as pimg:

            # trigger the exp ACT-table load (~2.7us) immediately so it
            # overlaps the input DMA instead of serializing after it
            warm = cpool.tile([SLOTS, 1], f32, tag="warm")
            nc.gpsimd.memset(warm, 0.0)
            nc.scalar.activation(warm, warm, Act.Exp)

            cf = cpool.tile([KB, CC], bf16, tag="coef")
            wt = cpool.tile([SLOTS, NG, WD], bf16, tag="wmat")
            nc.sync.dma_start(out=cf, in_=coef_d[:])
            nc.scalar.dma_start(out=wt, in_=w_d[:].rearrange("p (g w) -> p g w", w=WD))
            bt = cf[:, 0:PIX]

            sigs = []
            for g in range(NG):
                lh = cf[:, PIX + g * SLOTS:PIX + (g + 1) * SLOTS]
                sig = psig.tile([SLOTS, PIX], f32, tag="sig", name=f"sig{g}")
                nc.tensor.matmul(sig, lh, bt, start=True, stop=True)
                sigs.append(sig)
            for g in range(NG):
                alpha = apool.tile([SLOTS, PIX], bf16, tag="alpha")
                nc.scalar.activation(alpha, sigs[g], Act.Exp, scale=-1.0)
                wd = WDs[g]
                img = pimg.tile([wd, PIX], f32, tag="img", name=f"img{g}")
                nc.tensor.matmul(img, wt[:, g, 0:wd], alpha, start=True, stop=True)
                st = spool.tile([wd, PIX], bf16, tag="st", name=f"st{g}")
                nc.vector.tensor_scalar(out=st, in0=img, scalar1=0.0,
                                        scalar2=1.0, op0=Alu.max, op1=Alu.min)
                eng = nc.scalar if g == 1 else nc.sync
                eng.dma_start(out=out_d[0:wd, g * PIX:(g + 1) * PIX], in_=st)

    bass_rust.generate_event_semaphores(nc)
    return nc


def _plan(cx, cy, lam):
    """Tile binning + first-fit-decreasing slot packing (host side)."""
    r = np.sqrt(2.0 * CUT * np.maximum(lam, 0.0)) + 1.0
    tiles = []  # (t, ty, tx, index-array)
    for t in range(T):
        x0 = np.clip(((cx[t] - r[t]) // TILE).astype(int), 0, NT - 1)
        x1 = np.clip(((cx[t] + r[t]) // TILE).astype(int), 0, NT - 1)
        y0 = np.clip(((cy[t] - r[t]) // TILE).astype(int), 0, NT - 1)
        y1 = np.clip(((cy[t] + r[t]) // TILE).astype(int), 0, NT - 1)
        buckets = [[[] for _ in range(NT)] for _ in range(NT)]
        for n in range(N):
            for ty in range(y0[n], y1[n] + 1):
                for tx in range(x0[n], x1[n] + 1):
                    buckets[ty][tx].append(n)
        for ty in range(NT):
            for tx in range(NT):
                if buckets[ty][tx]:
                    assert len(buckets[ty][tx]) <= SLOTS
                    tiles.append((t, ty, tx, np.asarray(buckets[ty][tx])))

    tiles.sort(key=lambda e: -len(e[3]))
    bins = []  # [slots_used, [(t,ty,tx,idxs,slot_off), ...]]
    for t, ty, tx, idxs in tiles:
        n = len(idxs)
        for b in bins:
            if b[0] + n <= SLOTS and len(b[1]) < GMAX:
                b[1].append((t, ty, tx, idxs, b[0]))
                b[0] += n
                break
        else:
            bins.append([n, [(t, ty, tx, idxs, 0)]])
    return bins


def _ensure_ntff_hook():
    """Provide antenv.axon_hooks (missing in this image) so trace=True works."""
    import sys, types, ctypes, contextlib
    if "antenv.axon_hooks" in sys.modules:
        return
    so_path = "/opt/axon/libaxon_pjrt.so"
    if not os.path.exists(so_path):
        return
    lib = ctypes.CDLL(so_path)
    if not hasattr(lib, "axon_start_nrt_profile"):
        return
    lib.axon_start_nrt_profile.argtypes = [ctypes.POINTER(ctypes.c_int64), ctypes.c_size_t]
    lib.axon_start_nrt_profile.restype = ctypes.c_int64
    lib.axon_stop_nrt_profile.argtypes = [ctypes.c_char_p]
    lib.axon_stop_nrt_profile.restype = ctypes.c_int64

    import contextlib

    @contextlib.contextmanager
    def _hook(output_dir, device_ids):
        import jax
        jax.devices()
        if device_ids:
            ids = (ctypes.c_int64 * len(device_ids))(*device_ids)
            rc = lib.axon_start_nrt_profile(ids, len(device_ids))
        else:
            rc = lib.axon_start_nrt_profile(None, 0)
        if rc != 0:
            raise RuntimeError(f"axon_start_nrt_profile rc={rc}")
        try:
            yield
        finally:
            n = lib.axon_stop_nrt_profile(str(output_dir).encode())
            print(f"profile: {n} file(s) written to {output_dir}")

    mod = types.ModuleType("antenv.axon_hooks")
    mod.get_axon_ntff_profile_hook = lambda: _hook
    mod.set_axon_ntff_profile_hook = lambda h: None
    sys.modules["antenv.axon_hooks"] = mod


def kernel(xyz, cholesky, opacity, features_dc):
    from concourse import bass_utils

    xyz = np.asarray(xyz, np.float32)
    cholesky = np.asarray(cholesky, np.float32)
    opacity = np.asarray(opacity, np.float32)
    features_dc = np.asarray(features_dc, np.float32)

    # host-side gaussian math, float64
    means = np.tanh(xyz.astype(np.float64))
    cx = 0.5 * W * (means[..., 0] + 1.0)
    cy = 0.5 * H * (means[..., 1] + 1.0)
    chol = cholesky.astype(np.float64) + np.array([0.5, 0.0, 0.5])
    l0, l1, l2 = chol[..., 0], chol[..., 1], chol[..., 2]
    sxx, sxy, syy = l0 * l0, l0 * l1, l1 * l1 + l2 * l2
    det = sxx * syy - sxy * sxy
    ca, cb, cc = syy / det, -sxy / det, sxx / det
    tr = sxx + syy
    lam = tr / 2 + np.sqrt(np.maximum(tr * tr / 4 - det, 0.0))
    colors = 1.0 / (1.0 + np.exp(-features_dc.astype(np.float64)))   # (N,3)
    opac = (1.0 / (1.0 + np.exp(-opacity.astype(np.float64))))[:, 0]  # (N,)

    bins = _plan(cx, cy, lam)
    bins.sort(key=lambda b: -len(b[1]))   # largest tile-count first
    B = len(bins)
    NG = (B + N_CORES - 1) // N_CORES
    CC = PIX + NG * SLOTS
    # per-slot output widths: slot g serves bins[g*8 .. g*8+7] (dealt below)
    WDs = tuple(3 * max((len(bins[g * N_CORES + c][1])
                         for c in range(N_CORES) if g * N_CORES + c < B), default=1)
                for g in range(NG))

    # shared basis: tile-local integer-centered pixel coords (exact in bf16)
    c0 = float(TILE // 2)                                   # 8.0
    gxl = ((np.arange(PIX) % TILE) - c0).astype(np.float64)
    gyl = ((np.arange(PIX) // TILE) - c0).astype(np.float64)
    basis = np.stack([gxl * gxl, gxl * gxl, gxl * gyl, gyl * gyl, gyl * gyl,
                      gxl, gxl, gyl, gyl])                  # (9,PIX)

    in_maps = []
    meta = []  # per core: per bin: list of (t,ty,tx,e)
    for c in range(N_CORES):
        coef = np.zeros((KB, CC), np.float64)
        coef[:, 0:PIX] = basis
        wmat = np.zeros((SLOTS, NG * WD), np.float64)
        core_meta = []
        for g in range(NG):
            k = g * N_CORES + c
            ents = []
            if k < B:
                for e, (t, ty, tx, idxs, off) in enumerate(bins[k][1]):
                    n = len(idxs)
                    ex = cx[t, idxs] - (tx * TILE + c0)
                    ey = cy[t, idxs] - (ty * TILE + c0)
                    A_ = 0.5 * ca[t, idxs]
                    B_ = cb[t, idxs]
                    C_ = 0.5 * cc[t, idxs]
                    D_ = -(ca[t, idxs] * ex + cb[t, idxs] * ey)
                    E_ = -(cb[t, idxs] * ex + cc[t, idxs] * ey)
                    F_ = (0.5 * ca[t, idxs] * ex * ex + cb[t, idxs] * ex * ey
                          + 0.5 * cc[t, idxs] * ey * ey)
                    Di, Ei = np.round(D_), np.round(E_)
                    sl = slice(PIX + g * SLOTS + off, PIX + g * SLOTS + off + n)
                    coef[:, sl] = np.stack([
                        np.full(n, 0.25), -(0.25 - A_), B_,
                        np.full(n, 0.25), -(0.25 - C_),
                        Di, D_ - Di, Ei, E_ - Ei])
                    wmat[off:off + n, g * WD + 3 * e:g * WD + 3 * e + 3] = \
                        colors[idxs] * (opac[idxs] * np.exp(-F_))[:, None]
                    ents.append((t, ty, tx, e))
            core_meta.append(ents)
        in_maps.append({"coef": coef.astype(ml_dtypes.bfloat16),
                        "wmat": wmat.astype(ml_dtypes.bfloat16)})
        meta.append(core_meta)

    key = (NG, WDs)
    if key not in _CACHE:
        _CACHE[key] = _build_nc(NG, WDs)
    nc = _CACHE[key]

    trace = bool(int(os.environ.get("GS_TRACE", "0")))
    if trace:
        _ensure_ntff_hook()
    res = bass_utils.run_bass_kernel_spmd(
        nc, in_maps, core_ids=list(range(N_CORES)), trace=trace)
    kernel.last_result = res

    img = np.zeros((T, 3, H, W), np.float32)
    for c in range(N_CORES):
        o = np.asarray(res.results[c]["out"]).astype(np.float32).reshape(WD, NG, PIX)
        for g, ents in enumerate(meta[c]):
            for (t, ty, tx, e) in ents:
                img[t, :, ty * TILE:(ty + 1) * TILE, tx * TILE:(tx + 1) * TILE] = \
                    o[3 * e:3 * e + 3, g].reshape(3, TILE, TILE)
    return img


# revision 10
# speedup vs baseline: 1.0511x; 1.0327x over previous
"""GaussianImage (Cholesky) renderer on 8 trn2 NeuronCores.

Strategy: tile-parallel over the pixel grid with slot packing.  The
256x256 image is cut into 16x16-pixel tiles.  The host does all the
per-gaussian math (tanh / sigmoid / conic / quadratic-form coefficients,
in float64) and bins gaussians to the tiles they can reach (conservative
support radius; beyond it exp(-sigma) < exp(-CUT) is dropped).  Each
(gaussian, tile) pair is one SLOT; slots pack 128-per-bin across up to
GMAX tiles.  Key fact: every tile uses the SAME local pixel window, so
sigma for a whole bin is ONE K=9 matmul against a shared 9x256 basis:

  sigma[s,p] = 0.25*gx2[p] - Ar[s]*gx2[p] + B[s]*gxy[p] + 0.25*gy2[p]
             - Cr[s]*gy2[p] + Di[s]*gx[p] + Df[s]*gx[p] + Ei[s]*gy[p]
             + Ef[s]*gy[p]

with gx,gy integer-centered (exact in bf16), quadratic coefs split as
0.25 - residual and linear coefs split int+frac, so the bf16 matmul
carries < 0.02 absolute error in sigma.  The constant term F of each
quadratic form is folded into the opacity host-side
(w' = color * opac * exp(-F)); |F| <= ~82 keeps exponents in range.

  alpha = Exp(-sigma)            [ScalarE, -> bf16]
  img   = W'^T @ alpha           [TensorE bf16, W' slot-block columns]
  out   = clamp(img, 0, 1)       [VectorE, also the PSUM->SBUF move]

All matmuls are bf16.  Instruction/semaphore count dominates the
runtime at this size, not FLOPs or bytes, so the device program is raw
bass (no TileContext): manual semaphores, no buffer reuse, two input
DMAs split across the two HWDGE rings, one merged output DMA.  Each
pixel is owned by exactly one tile -> no cross-core reduction.  Per
core: NG(=3) bins of sigma-matmul -> exp -> img-matmul -> clamp.
"""

import os
import numpy as np
import ml_dtypes

T, N, H, W = 2, 512, 256, 256
TILE = 16
NT = H // TILE            # 16 tiles per axis
PIX = TILE * TILE         # 256
N_CORES = 8
SLOTS = 128               # gaussian-slot capacity of one bin (partition dim)
GMAX = 25                 # max tiles per bin (3 color rows each -> WD = 75)
KB = 9                    # basis rows
WD = 3 * GMAX             # 75 output color rows per bin
CUT = 4.0                 # sigma cutoff for binning

_CACHE = {}


def _build_nc_raw(NG, WDs):
    import concourse.bass as bass
    import concourse.mybir as mybir
    import bass_rust

    f32 = mybir.dt.float32
    bf16 = mybir.dt.bfloat16
    Alu = mybir.AluOpType
    Act = mybir.ActivationFunctionType

    CC = PIX + NG * SLOTS             # coef-tensor columns (basis + per-bin lhsT)
    nc = bass.Bass("TRN2")
    coef_d = nc.dram_tensor("coef", [KB, CC], bf16, kind="ExternalInput")
    w_d = nc.dram_tensor("wmat", [SLOTS, NG * WD], bf16, kind="ExternalInput")
    out_d = nc.dram_tensor("out", [WD, NG * PIX], bf16, kind="ExternalOutput")

    # Raw bass, no TileContext: manual semaphores + no buffer reuse.  The
    # NEFF epilogue zeroes the semaphore file per-engine right after each
    # engine's stream; with no final all-engine barrier, TensorE's ~6us
    # zeroing overlaps the output DMAs instead of serializing after them.
    # Safety: every semaphore we use must lie in VectorE's zeroing slice
    # [156, 206], and VectorE's stream must end by waiting for the output
    # DMAs to land (so no other engine zeroes a semaphore still in flight).
    sems = []
    tries = 0
    while len(sems) < 9:
        s = nc.alloc_semaphore(f"k{tries}")
        tries += 1
        assert tries < 250
        if s.num >= 156:
            assert s.num <= 206, f"sem {s.num} outside VectorE zeroing slice"
            sems.append(s)
    S_CF, S_WM, S_MM, S_AL, S_CL, S_O0, S_O1, S_O2, _ = sems
    S_O = [S_O0, S_O1, S_O2]

    cf = nc.alloc_sbuf_tensor("cf", [KB, CC], bf16)
    wt = nc.alloc_sbuf_tensor("wt", [SLOTS, NG * WD], bf16)
    warm = nc.alloc_sbuf_tensor("warm", [SLOTS, 1], f32)
    sig = [nc.alloc_psum_tensor(f"sig{g}", [SLOTS, PIX], f32) for g in range(NG)]
    al = [nc.alloc_sbuf_tensor(f"al{g}", [SLOTS, PIX], bf16) for g in range(NG)]
    img = [nc.alloc_psum_tensor(f"img{g}", [WDs[g], PIX], f32) for g in range(NG)]
    stm = nc.alloc_sbuf_tensor("stm", [WD, NG, PIX], bf16)
    zero_ap = nc.const_aps.aps[(f32, 0.0)]

    # SYNC: coef in; out0/out2 after their clamps
    nc.sync.dma_start(out=cf[:], in_=coef_d[:]).then_inc(S_CF, 16)
    # SCALAR: wmat in (2nd HWDGE ring), exp-table warm, the three exps, out1
    nc.scalar.dma_start(out=wt[:], in_=w_d[:]).then_inc(S_WM, 16)
    nc.scalar.activation(warm[:], zero_ap, Act.Exp)

    bt = cf[:, 0:PIX]
    # TENSOR: three sigma matmuls, then the three img matmuls
    nc.tensor.wait_ge(S_CF, 16)
    for g in range(NG):
        lh = cf[:, PIX + g * SLOTS:PIX + (g + 1) * SLOTS]
        nc.tensor.matmul(sig[g][:], lh, bt, start=True, stop=True).then_inc(S_MM, 1)
    for g in range(NG):
        nc.scalar.wait_ge(S_MM, g + 1)
        nc.scalar.activation(al[g][:], sig[g][:], Act.Exp, scale=-1.0).then_inc(S_AL, 1)
    nc.tensor.wait_ge(S_WM, 16)
    for g in range(NG):
        nc.tensor.wait_ge(S_AL, g + 1)
        nc.tensor.matmul(img[g][:], wt[:, g * WD:g * WD + WDs[g]], al[g][:],
                         start=True, stop=True).then_inc(S_MM, 1)
    # VECTOR: clamps into one staging tensor; then the DMA-landing guard
    for g in range(NG):
        nc.vector.wait_ge(S_MM, NG + g + 1)
        nc.vector.tensor_scalar(out=stm[0:WDs[g], g, :], in0=img[g][:], scalar1=0.0,
                                scalar2=1.0, op0=Alu.max, op1=Alu.min).then_inc(S_CL, 1)
    # single merged out DMA on the sync ring
    nc.sync.wait_ge(S_CL, NG)
    nc.sync.dma_start(out=out_d[:], in_=stm[:].rearrange("w g p -> w (g p)")).then_inc(S_O0, 16)
    nc.vector.wait_ge(S_O0, 16)

    bass_rust.generate_event_semaphores(nc)
    return nc


def _build_nc_tile(NG, WDs):
    import concourse.bass as bass
    import concourse.mybir as mybir
    from concourse.tile import TileContext
    import bass_rust

    f32 = mybir.dt.float32
    bf16 = mybir.dt.bfloat16
    Alu = mybir.AluOpType
    Act = mybir.ActivationFunctionType

    CC = PIX + NG * SLOTS             # coef-tensor columns (basis + per-bin lhsT)
    nc = bass.Bass("TRN2")
    coef_d = nc.dram_tensor("coef", [KB, CC], bf16, kind="ExternalInput")
    w_d = nc.dram_tensor("wmat", [SLOTS, NG * WD], bf16, kind="ExternalInput")
    out_d = nc.dram_tensor("out", [WD, NG * PIX], bf16, kind="ExternalOutput")

    with TileContext(nc) as tc:
        with tc.tile_pool(name="const", bufs=1) as cpool, \
             tc.tile_pool(name="alpha", bufs=3) as apool, \
             tc.tile_pool(name="stg", bufs=3) as spool, \
             tc.tile_pool(name="ps_sig", bufs=3, space="PSUM") as psig, \
             tc.tile_pool(name="ps_img", bufs=2, space="PSUM") as pimg:

            # trigger the exp ACT-table load (~2.7us) immediately so it
            # overlaps the input DMA instead of serializing after it
            warm = cpool.tile([SLOTS, 1], f32, tag="warm")
            nc.gpsimd.memset(warm, 0.0)
            nc.scalar.activation(warm, warm, Act.Exp)

            cf = cpool.tile([KB, CC], bf16, tag="coef")
            wt = cpool.tile([SLOTS, NG, WD], bf16, tag="wmat")
            nc.sync.dma_start(out=cf, in_=coef_d[:])
            nc.scalar.dma_start(out=wt, in_=w_d[:].rearrange("p (g w) -> p g w", w=WD))
            bt = cf[:, 0:PIX]

            sigs = []
            for g in range(NG):
                lh = cf[:, PIX + g * SLOTS:PIX + (g + 1) * SLOTS]
                sig = psig.tile([SLOTS, PIX], f32, tag="sig", name=f"sig{g}")
                nc.tensor.matmul(sig, lh, bt, start=True, stop=True)
                sigs.append(sig)
            for g in range(NG):
                alpha = apool.tile([SLOTS, PIX], bf16, tag="alpha")
                nc.scalar.activation(alpha, sigs[g], Act.Exp, scale=-1.0)
                wd = WDs[g]
                img = pimg.tile([wd, PIX], f32, tag="img", name=f"img{g}")
                nc.tensor.matmul(img, wt[:, g, 0:wd], alpha, start=True, stop=True)
                st = spool.tile([wd, PIX], bf16, tag="st", name=f"st{g}")
                nc.vector.tensor_scalar(out=st, in0=img, scalar1=0.0,
                                        scalar2=1.0, op0=Alu.max, op1=Alu.min)
                eng = nc.scalar if g == 1 else nc.sync
                eng.dma_start(out=out_d[0:wd, g * PIX:(g + 1) * PIX], in_=st)

    bass_rust.generate_event_semaphores(nc)
    return nc


def _build_nc(NG, WDs):
    try:
        return _build_nc_raw(NG, WDs)
    except AssertionError:
        return _build_nc_tile(NG, WDs)


def _plan(cx, cy, lam):
    """Tile binning + first-fit-decreasing slot packing (host side)."""
    r = np.sqrt(2.0 * CUT * np.maximum(lam, 0.0)) + 1.0
    tiles = []  # (t, ty, tx, index-array)
    for t in range(T):
        x0 = np.clip(((cx[t] - r[t]) // TILE).astype(int), 0, NT - 1)
        x1 = np.clip(((cx[t] + r[t]) // TILE).astype(int), 0, NT - 1)
        y0 = np.clip(((cy[t] - r[t]) // TILE).astype(int), 0, NT - 1)
        y1 = np.clip(((cy[t] + r[t]) // TILE).astype(int), 0, NT - 1)
        buckets = [[[] for _ in range(NT)] for _ in range(NT)]
        for n in range(N):
            for ty in range(y0[n], y1[n] + 1):
                for tx in range(x0[n], x1[n] + 1):
                    buckets[ty][tx].append(n)
        for ty in range(NT):
            for tx in range(NT):
                if buckets[ty][tx]:
                    assert len(buckets[ty][tx]) <= SLOTS
                    tiles.append((t, ty, tx, np.asarray(buckets[ty][tx])))

    tiles.sort(key=lambda e: -len(e[3]))
    bins = []  # [slots_used, [(t,ty,tx,idxs,slot_off), ...]]
    for t, ty, tx, idxs in tiles:
        n = len(idxs)
        for b in bins:
            if b[0] + n <= SLOTS and len(b[1]) < GMAX:
                b[1].append((t, ty, tx, idxs, b[0]))
                b[0] += n
                break
        else:
            bins.append([n, [(t, ty, tx, idxs, 0)]])
    return bins


def _ensure_ntff_hook():
    """Provide antenv.axon_hooks (missing in this image) so trace=True works."""
    import sys, types, ctypes, contextlib
    if "antenv.axon_hooks" in sys.modules:
        return
    so_path = "/opt/axon/libaxon_pjrt.so"
    if not os.path.exists(so_path):
        return
    lib = ctypes.CDLL(so_path)
    if not hasattr(lib, "axon_start_nrt_profile"):
        return
    lib.axon_start_nrt_profile.argtypes = [ctypes.POINTER(ctypes.c_int64), ctypes.c_size_t]
    lib.axon_start_nrt_profile.restype = ctypes.c_int64
    lib.axon_stop_nrt_profile.argtypes = [ctypes.c_char_p]
    lib.axon_stop_nrt_profile.restype = ctypes.c_int64

    import contextlib

    @contextlib.contextmanager
    def _hook(output_dir, device_ids):
        import jax
        jax.devices()
        if device_ids:
            ids = (ctypes.c_int64 * len(device_ids))(*device_ids)
            rc = lib.axon_start_nrt_profile(ids, len(device_ids))
        else:
            rc = lib.axon_start_nrt_profile(None, 0)
        if rc != 0:
            raise RuntimeError(f"axon_start_nrt_profile rc={rc}")
        try:
            yield
        finally:
            n = lib.axon_stop_nrt_profile(str(output_dir).encode())
            print(f"profile: {n} file(s) written to {output_dir}")

    mod = types.ModuleType("antenv.axon_hooks")
    mod.get_axon_ntff_profile_hook = lambda: _hook
    mod.set_axon_ntff_profile_hook = lambda h: None
    sys.modules["antenv.axon_hooks"] = mod


def kernel(xyz, cholesky, opacity, features_dc):
    from concourse import bass_utils

    xyz = np.asarray(xyz, np.float32)
    cholesky = np.asarray(cholesky, np.float32)
    opacity = np.asarray(opacity, np.float32)
    features_dc = np.asarray(features_dc, np.float32)

    # host-side gaussian math, float64
    means = np.tanh(xyz.astype(np.float64))
    cx = 0.5 * W * (means[..., 0] + 1.0)
    cy = 0.5 * H * (means[..., 1] + 1.0)
    chol = cholesky.astype(np.float64) + np.array([0.5, 0.0, 0.5])
    l0, l1, l2 = chol[..., 0], chol[..., 1], chol[..., 2]
    sxx, sxy, syy = l0 * l0, l0 * l1, l1 * l1 + l2 * l2
    det = sxx * syy - sxy * sxy
    ca, cb, cc = syy / det, -sxy / det, sxx / det
    tr = sxx + syy
    lam = tr / 2 + np.sqrt(np.maximum(tr * tr / 4 - det, 0.0))
    colors = 1.0 / (1.0 + np.exp(-features_dc.astype(np.float64)))   # (N,3)
    opac = (1.0 / (1.0 + np.exp(-opacity.astype(np.float64))))[:, 0]  # (N,)

    bins = _plan(cx, cy, lam)
    bins.sort(key=lambda b: -len(b[1]))   # largest tile-count first
    B = len(bins)
    NG = (B + N_CORES - 1) // N_CORES
    CC = PIX + NG * SLOTS
    # per-slot output widths: slot g serves bins[g*8 .. g*8+7] (dealt below)
    WDs = tuple(3 * max((len(bins[g * N_CORES + c][1])
                         for c in range(N_CORES) if g * N_CORES + c < B), default=1)
                for g in range(NG))

    # shared basis: tile-local integer-centered pixel coords (exact in bf16)
    c0 = float(TILE // 2)                                   # 8.0
    gxl = ((np.arange(PIX) % TILE) - c0).astype(np.float64)
    gyl = ((np.arange(PIX) // TILE) - c0).astype(np.float64)
    basis = np.stack([gxl * gxl, gxl * gxl, gxl * gyl, gyl * gyl, gyl * gyl,
                      gxl, gxl, gyl, gyl])                  # (9,PIX)

    in_maps = []
    meta = []  # per core: per bin: list of (t,ty,tx,e)
    for c in range(N_CORES):
        coef = np.zeros((KB, CC), np.float64)
        coef[:, 0:PIX] = basis
        wmat = np.zeros((SLOTS, NG * WD), np.float64)
        core_meta = []
        for g in range(NG):
            k = g * N_CORES + c
            ents = []
            if k < B:
                for e, (t, ty, tx, idxs, off) in enumerate(bins[k][1]):
                    n = len(idxs)
                    ex = cx[t, idxs] - (tx * TILE + c0)
                    ey = cy[t, idxs] - (ty * TILE + c0)
                    A_ = 0.5 * ca[t, idxs]
                    B_ = cb[t, idxs]
                    C_ = 0.5 * cc[t, idxs]
                    D_ = -(ca[t, idxs] * ex + cb[t, idxs] * ey)
                    E_ = -(cb[t, idxs] * ex + cc[t, idxs] * ey)
                    F_ = (0.5 * ca[t, idxs] * ex * ex + cb[t, idxs] * ex * ey
                          + 0.5 * cc[t, idxs] * ey * ey)
                    Di, Ei = np.round(D_), np.round(E_)
                    sl = slice(PIX + g * SLOTS + off, PIX + g * SLOTS + off + n)
                    coef[:, sl] = np.stack([
                        np.full(n, 0.25), -(0.25 - A_), B_,
                        np.full(n, 0.25), -(0.25 - C_),
                        Di, D_ - Di, Ei, E_ - Ei])
                    wmat[off:off + n, g * WD + 3 * e:g * WD + 3 * e + 3] = \
                        colors[idxs] * (opac[idxs] * np.exp(-F_))[:, None]
                    ents.append((t, ty, tx, e))
            core_meta.append(ents)
        in_maps.append({"coef": coef.astype(ml_dtypes.bfloat16),
                        "wmat": wmat.astype(ml_dtypes.bfloat16)})
        meta.append(core_meta)

    key = (NG, WDs)
    if key not in _CACHE:
        _CACHE[key] = _build_nc(NG, WDs)
    nc = _CACHE[key]

    trace = bool(int(os.environ.get("GS_TRACE", "0")))
    if trace:
        _ensure_ntff_hook()
    res = bass_utils.run_bass_kernel_spmd(
        nc, in_maps, core_ids=list(range(N_CORES)), trace=trace)
    kernel.last_result = res

    img = np.zeros((T, 3, H, W), np.float32)
    for c in range(N_CORES):
        o = np.asarray(res.results[c]["out"]).astype(np.float32).reshape(WD, NG, PIX)
        for g, ents in enumerate(meta[c]):
            for (t, ty, tx, e) in ents:
                img[t, :, ty * TILE:(ty + 1) * TILE, tx * TILE:(tx + 1) * TILE] = \
                    o[3 * e:3 * e + 3, g].reshape(3, TILE, TILE)
    return img


# revision 11
# speedup vs baseline: 1.0564x; 1.0050x over previous
"""GaussianImage (Cholesky) renderer on 8 trn2 NeuronCores.

Strategy: tile-parallel over the pixel grid with slot packing.  The
256x256 image is cut into 16x16-pixel tiles.  The host does all the
per-gaussian math (tanh / sigmoid / conic / quadratic-form coefficients,
in float64) and bins gaussians to the tiles they can reach (conservative
support radius; beyond it exp(-sigma) < exp(-CUT) is dropped).  Each
(gaussian, tile) pair is one SLOT; slots pack 128-per-bin across up to
GMAX tiles.  Key fact: every tile uses the SAME local pixel window, so
sigma for a whole bin is ONE K=9 matmul against a shared 9x256 basis:

  sigma[s,p] = 0.25*gx2[p] - Ar[s]*gx2[p] + B[s]*gxy[p] + 0.25*gy2[p]
             - Cr[s]*gy2[p] + Di[s]*gx[p] + Df[s]*gx[p] + Ei[s]*gy[p]
             + Ef[s]*gy[p]

with gx,gy integer-centered (exact in bf16), quadratic coefs split as
0.25 - residual and linear coefs split int+frac, so the bf16 matmul
carries < 0.02 absolute error in sigma.  The constant term F of each
quadratic form is folded into the opacity host-side
(w' = color * opac * exp(-F)); |F| <= ~82 keeps exponents in range.

  alpha = Exp(-sigma)            [ScalarE, -> bf16]
  img   = W'^T @ alpha           [TensorE bf16, W' slot-block columns]
  out   = clamp(img, 0, 1)       [VectorE, also the PSUM->SBUF move]

All matmuls are bf16.  Instruction/semaphore count dominates the
runtime at this size, not FLOPs or bytes, so the device program is raw
bass (no TileContext): manual semaphores, no buffer reuse, two input
DMAs split across the two HWDGE rings, one merged output DMA.  Each
pixel is owned by exactly one tile -> no cross-core reduction.  Per
core: NG(=3) bins of sigma-matmul -> exp -> img-matmul -> clamp.
"""

import os
import numpy as np
import ml_dtypes

T, N, H, W = 2, 512, 256, 256
TILE = 16
NT = H // TILE            # 16 tiles per axis
PIX = TILE * TILE         # 256
N_CORES = 8
SLOTS = 128               # gaussian-slot capacity of one bin (partition dim)
GMAX = 25                 # max tiles per bin (3 color rows each -> WD = 75)
KB = 9                    # basis rows
WD = 3 * GMAX             # 75 output color rows per bin
CUT = 4.0                 # sigma cutoff for binning

_CACHE = {}


def _build_nc_raw(NG, WDs):
    import concourse.bass as bass
    import concourse.mybir as mybir
    import bass_rust

    f32 = mybir.dt.float32
    bf16 = mybir.dt.bfloat16
    Alu = mybir.AluOpType
    Act = mybir.ActivationFunctionType

    CC = PIX + NG * SLOTS             # coef-tensor columns (basis + per-bin lhsT)
    nc = bass.Bass("TRN2")
    coef_d = nc.dram_tensor("coef", [KB, CC], bf16, kind="ExternalInput")
    w_d = nc.dram_tensor("wmat", [SLOTS, NG * WD], bf16, kind="ExternalInput")
    out_d = nc.dram_tensor("out", [WD, NG * PIX], bf16, kind="ExternalOutput")

    # Raw bass, no TileContext: manual semaphores + no buffer reuse.  The
    # NEFF epilogue zeroes the semaphore file per-engine right after each
    # engine's stream; with no final all-engine barrier, TensorE's ~6us
    # zeroing overlaps the output DMAs instead of serializing after them.
    # Safety: every semaphore we use must lie in VectorE's zeroing slice
    # [156, 206], and VectorE's stream must end by waiting for the output
    # DMAs to land (so no other engine zeroes a semaphore still in flight).
    sems = []
    tries = 0
    while len(sems) < 9:
        s = nc.alloc_semaphore(f"k{tries}")
        tries += 1
        assert tries < 250
        if s.num >= 156:
            assert s.num <= 206, f"sem {s.num} outside VectorE zeroing slice"
            sems.append(s)
    S_CF, S_WM, S_MM, S_AL, S_CL, S_O0, S_O1, S_O2, _ = sems
    S_O = [S_O0, S_O1, S_O2]

    cf = nc.alloc_sbuf_tensor("cf", [KB, CC], bf16)
    wt = nc.alloc_sbuf_tensor("wt", [SLOTS, NG * WD], bf16)
    warm = nc.alloc_sbuf_tensor("warm", [SLOTS, 1], f32)
    sig = [nc.alloc_psum_tensor(f"sig{g}", [SLOTS, PIX], f32) for g in range(NG)]
    al = [nc.alloc_sbuf_tensor(f"al{g}", [SLOTS, PIX], bf16) for g in range(NG)]
    img = [nc.alloc_psum_tensor(f"img{g}", [WDs[g], PIX], f32) for g in range(NG)]
    stm = nc.alloc_sbuf_tensor("stm", [WD, NG, PIX], bf16)
    zero_ap = nc.const_aps.aps[(f32, 0.0)]

    # SYNC: coef in; out0/out2 after their clamps
    nc.sync.dma_start(out=cf[:], in_=coef_d[:]).then_inc(S_CF, 16)
    # SCALAR: wmat in (2nd HWDGE ring), exp-table warm, the three exps, out1
    nc.scalar.dma_start(out=wt[:], in_=w_d[:]).then_inc(S_WM, 16)
    nc.scalar.activation(warm[:], zero_ap, Act.Exp)

    bt = cf[:, 0:PIX]
    # TENSOR: three sigma matmuls, then the three img matmuls
    nc.tensor.wait_ge(S_CF, 16)
    for g in range(NG):
        lh = cf[:, PIX + g * SLOTS:PIX + (g + 1) * SLOTS]
        nc.tensor.matmul(sig[g][:], lh, bt, start=True, stop=True).then_inc(S_MM, 1)
    for g in range(NG):
        nc.scalar.wait_ge(S_MM, g + 1)
        nc.scalar.activation(al[g][:], sig[g][:], Act.Exp, scale=-1.0).then_inc(S_AL, 1)
    nc.tensor.wait_ge(S_WM, 16)
    for g in range(NG):
        nc.tensor.wait_ge(S_AL, g + 1)
        nc.tensor.matmul(img[g][:], wt[:, g * WD:g * WD + WDs[g]], al[g][:],
                         start=True, stop=True).then_inc(S_MM, 1)
    # VECTOR: clamps into one staging tensor; then the DMA-landing guard
    for g in range(NG):
        nc.vector.wait_ge(S_MM, NG + g + 1)
        nc.vector.tensor_scalar(out=stm[0:WDs[g], g, :], in0=img[g][:], scalar1=0.0,
                                scalar2=1.0, op0=Alu.max, op1=Alu.min).then_inc(S_CL, 1)
    # single merged out DMA on the sync ring
    nc.sync.wait_ge(S_CL, NG)
    nc.sync.dma_start(out=out_d[:], in_=stm[:].rearrange("w g p -> w (g p)")).then_inc(S_O0, 16)
    nc.vector.wait_ge(S_O0, 16)

    bass_rust.generate_event_semaphores(nc)
    return nc


def _build_nc_tile(NG, WDs):
    import concourse.bass as bass
    import concourse.mybir as mybir
    from concourse.tile import TileContext
    import bass_rust

    f32 = mybir.dt.float32
    bf16 = mybir.dt.bfloat16
    Alu = mybir.AluOpType
    Act = mybir.ActivationFunctionType

    CC = PIX + NG * SLOTS             # coef-tensor columns (basis + per-bin lhsT)
    nc = bass.Bass("TRN2")
    coef_d = nc.dram_tensor("coef", [KB, CC], bf16, kind="ExternalInput")
    w_d = nc.dram_tensor("wmat", [SLOTS, NG * WD], bf16, kind="ExternalInput")
    out_d = nc.dram_tensor("out", [WD, NG * PIX], bf16, kind="ExternalOutput")

    with TileContext(nc) as tc:
        with tc.tile_pool(name="const", bufs=1) as cpool, \
             tc.tile_pool(name="alpha", bufs=3) as apool, \
             tc.tile_pool(name="stg", bufs=3) as spool, \
             tc.tile_pool(name="ps_sig", bufs=3, space="PSUM") as psig, \
             tc.tile_pool(name="ps_img", bufs=2, space="PSUM") as pimg:

            # trigger the exp ACT-table load (~2.7us) immediately so it
            # overlaps the input DMA instead of serializing after it
            warm = cpool.tile([SLOTS, 1], f32, tag="warm")
            nc.gpsimd.memset(warm, 0.0)
            nc.scalar.activation(warm, warm, Act.Exp)

            cf = cpool.tile([KB, CC], bf16, tag="coef")
            wt = cpool.tile([SLOTS, NG, WD], bf16, tag="wmat")
            nc.sync.dma_start(out=cf, in_=coef_d[:])
            nc.scalar.dma_start(out=wt, in_=w_d[:].rearrange("p (g w) -> p g w", w=WD))
            bt = cf[:, 0:PIX]

            sigs = []
            for g in range(NG):
                lh = cf[:, PIX + g * SLOTS:PIX + (g + 1) * SLOTS]
                sig = psig.tile([SLOTS, PIX], f32, tag="sig", name=f"sig{g}")
                nc.tensor.matmul(sig, lh, bt, start=True, stop=True)
                sigs.append(sig)
            for g in range(NG):
                alpha = apool.tile([SLOTS, PIX], bf16, tag="alpha")
                nc.scalar.activation(alpha, sigs[g], Act.Exp, scale=-1.0)
                wd = WDs[g]
                img = pimg.tile([wd, PIX], f32, tag="img", name=f"img{g}")
                nc.tensor.matmul(img, wt[:, g, 0:wd], alpha, start=True, stop=True)
                st = spool.tile([wd, PIX], bf16, tag="st", name=f"st{g}")
                nc.vector.tensor_scalar(out=st, in0=img, scalar1=0.0,
                                        scalar2=1.0, op0=Alu.max, op1=Alu.min)
                eng = nc.scalar if g == 1 else nc.sync
                eng.dma_start(out=out_d[0:wd, g * PIX:(g + 1) * PIX], in_=st)

    bass_rust.generate_event_semaphores(nc)
    return nc


def _build_nc(NG, WDs):
    try:
        return _build_nc_raw(NG, WDs)
    except Exception:
        return _build_nc_tile(NG, WDs)


def _plan(cx, cy, lam):
    """Tile binning + first-fit-decreasing slot packing (host side)."""
    r = np.sqrt(2.0 * CUT * np.maximum(lam, 0.0)) + 1.0
    tiles = []  # (t, ty, tx, index-array)
    for t in range(T):
        x0 = np.clip(((cx[t] - r[t]) // TILE).astype(int), 0, NT - 1)
        x1 = np.clip(((cx[t] + r[t]) // TILE).astype(int), 0, NT - 1)
        y0 = np.clip(((cy[t] - r[t]) // TILE).astype(int), 0, NT - 1)
        y1 = np.clip(((cy[t] + r[t]) // TILE).astype(int), 0, NT - 1)
        buckets = [[[] for _ in range(NT)] for _ in range(NT)]
        for n in range(N):
            for ty in range(y0[n], y1[n] + 1):
                for tx in range(x0[n], x1[n] + 1):
                    buckets[ty][tx].append(n)
        for ty in range(NT):
            for tx in range(NT):
                if buckets[ty][tx]:
                    assert len(buckets[ty][tx]) <= SLOTS
                    tiles.append((t, ty, tx, np.asarray(buckets[ty][tx])))

    tiles.sort(key=lambda e: -len(e[3]))
    bins = []  # [slots_used, [(t,ty,tx,idxs,slot_off), ...]]
    for t, ty, tx, idxs in tiles:
        n = len(idxs)
        for b in bins:
            if b[0] + n <= SLOTS and len(b[1]) < GMAX:
                b[1].append((t, ty, tx, idxs, b[0]))
                b[0] += n
                break
        else:
            bins.append([n, [(t, ty, tx, idxs, 0)]])
    return bins


def _ensure_ntff_hook():
    """Provide antenv.axon_hooks (missing in this image) so trace=True works."""
    import sys, types, ctypes, contextlib
    if "antenv.axon_hooks" in sys.modules:
        return
    so_path = "/opt/axon/libaxon_pjrt.so"
    if not os.path.exists(so_path):
        return
    lib = ctypes.CDLL(so_path)
    if not hasattr(lib, "axon_start_nrt_profile"):
        return
    lib.axon_start_nrt_profile.argtypes = [ctypes.POINTER(ctypes.c_int64), ctypes.c_size_t]
    lib.axon_start_nrt_profile.restype = ctypes.c_int64
    lib.axon_stop_nrt_profile.argtypes = [ctypes.c_char_p]
    lib.axon_stop_nrt_profile.restype = ctypes.c_int64

    import contextlib

    @contextlib.contextmanager
    def _hook(output_dir, device_ids):
        import jax
        jax.devices()
        if device_ids:
            ids = (ctypes.c_int64 * len(device_ids))(*device_ids)
            rc = lib.axon_start_nrt_profile(ids, len(device_ids))
        else:
            rc = lib.axon_start_nrt_profile(None, 0)
        if rc != 0:
            raise RuntimeError(f"axon_start_nrt_profile rc={rc}")
        try:
            yield
        finally:
            n = lib.axon_stop_nrt_profile(str(output_dir).encode())
            print(f"profile: {n} file(s) written to {output_dir}")

    mod = types.ModuleType("antenv.axon_hooks")
    mod.get_axon_ntff_profile_hook = lambda: _hook
    mod.set_axon_ntff_profile_hook = lambda h: None
    sys.modules["antenv.axon_hooks"] = mod


def kernel(xyz, cholesky, opacity, features_dc):
    from concourse import bass_utils

    xyz = np.asarray(xyz, np.float32)
    cholesky = np.asarray(cholesky, np.float32)
    opacity = np.asarray(opacity, np.float32)
    features_dc = np.asarray(features_dc, np.float32)

    # host-side gaussian math, float64
    means = np.tanh(xyz.astype(np.float64))
    cx = 0.5 * W * (means[..., 0] + 1.0)
    cy = 0.5 * H * (means[..., 1] + 1.0)
    chol = cholesky.astype(np.float64) + np.array([0.5, 0.0, 0.5])
    l0, l1, l2 = chol[..., 0], chol[..., 1], chol[..., 2]
    sxx, sxy, syy = l0 * l0, l0 * l1, l1 * l1 + l2 * l2
    det = sxx * syy - sxy * sxy
    ca, cb, cc = syy / det, -sxy / det, sxx / det
    tr = sxx + syy
    lam = tr / 2 + np.sqrt(np.maximum(tr * tr / 4 - det, 0.0))
    colors = 1.0 / (1.0 + np.exp(-features_dc.astype(np.float64)))   # (N,3)
    opac = (1.0 / (1.0 + np.exp(-opacity.astype(np.float64))))[:, 0]  # (N,)

    bins = _plan(cx, cy, lam)
    bins.sort(key=lambda b: -len(b[1]))   # largest tile-count first
    B = len(bins)
    NG = (B + N_CORES - 1) // N_CORES
    CC = PIX + NG * SLOTS
    # per-slot output widths: slot g serves bins[g*8 .. g*8+7] (dealt below)
    WDs = tuple(3 * max((len(bins[g * N_CORES + c][1])
                         for c in range(N_CORES) if g * N_CORES + c < B), default=1)
                for g in range(NG))

    # shared basis: tile-local integer-centered pixel coords (exact in bf16)
    c0 = float(TILE // 2)                                   # 8.0
    gxl = ((np.arange(PIX) % TILE) - c0).astype(np.float64)
    gyl = ((np.arange(PIX) // TILE) - c0).astype(np.float64)
    basis = np.stack([gxl * gxl, gxl * gxl, gxl * gyl, gyl * gyl, gyl * gyl,
                      gxl, gxl, gyl, gyl])                  # (9,PIX)

    in_maps = []
    meta = []  # per core: per bin: list of (t,ty,tx,e)
    for c in range(N_CORES):
        coef = np.zeros((KB, CC), np.float64)
        coef[:, 0:PIX] = basis
        wmat = np.zeros((SLOTS, NG * WD), np.float64)
        core_meta = []
        for g in range(NG):
            k = g * N_CORES + c
            ents = []
            if k < B:
                for e, (t, ty, tx, idxs, off) in enumerate(bins[k][1]):
                    n = len(idxs)
                    ex = cx[t, idxs] - (tx * TILE + c0)
                    ey = cy[t, idxs] - (ty * TILE + c0)
                    A_ = 0.5 * ca[t, idxs]
                    B_ = cb[t, idxs]
                    C_ = 0.5 * cc[t, idxs]
                    D_ = -(ca[t, idxs] * ex + cb[t, idxs] * ey)
                    E_ = -(cb[t, idxs] * ex + cc[t, idxs] * ey)
                    F_ = (0.5 * ca[t, idxs] * ex * ex + cb[t, idxs] * ex * ey
                          + 0.5 * cc[t, idxs] * ey * ey)
                    Di, Ei = np.round(D_), np.round(E_)
                    sl = slice(PIX + g * SLOTS + off, PIX + g * SLOTS + off + n)
                    coef[:, sl] = np.stack([
                        np.full(n, 0.25), -(0.25 - A_), B_,
                        np.full(n, 0.25), -(0.25 - C_),
                        Di, D_ - Di, Ei, E_ - Ei])
                    wmat[off:off + n, g * WD + 3 * e:g * WD + 3 * e + 3] = \
                        colors[idxs] * (opac[idxs] * np.exp(-F_))[:, None]
                    ents.append((t, ty, tx, e))
            core_meta.append(ents)
        in_maps.append({"coef": coef.astype(ml_dtypes.bfloat16),
                        "wmat": wmat.astype(ml_dtypes.bfloat16)})
        meta.append(core_meta)

    key = (NG, WDs)
    if key not in _CACHE:
        _CACHE[key] = _build_nc(NG, WDs)
    nc = _CACHE[key]

    trace = bool(int(os.environ.get("GS_TRACE", "0")))
    if trace:
        _ensure_ntff_hook()
    res = bass_utils.run_bass_kernel_spmd(
        nc, in_maps, core_ids=list(range(N_CORES)), trace=trace)
    kernel.last_result = res

    img = np.zeros((T, 3, H, W), np.float32)
    for c in range(N_CORES):
        o = np.asarray(res.results[c]["out"]).astype(np.float32).reshape(WD, NG, PIX)
        for g, ents in enumerate(meta[c]):
            for (t, ty, tx, e) in ents:
                img[t, :, ty * TILE:(ty + 1) * TILE, tx * TILE:(tx + 1) * TILE] = \
                    o[3 * e:3 * e + 3, g].reshape(3, TILE, TILE)
    return img
